# revision 51
# baseline (speedup 1.0000x reference)
"""Trainium2 Bass kernel for nn_HeteroForecastSageConv.

Strategy (8 NeuronCores, SPMD, degree-grouped edge streams):
 - Target (destination) nodes are relabeled by lexsorting on their
   (s2d, d2s, ct) in-degrees and chunking 128 consecutive nodes per block;
   blocks are dealt round-robin over the 8 cores.  Within most blocks the
   per-direction degree is constant, so that (block, direction) cell lays
   its edges out as full 128-wide tiles where slot j of tile r holds the
   r-th edge of destination j: the routing matrix is the identity and the
   segment sum is a plain PSUM accumulation of feature-major tiles
   (pretransform with stationary Wp at N<=512).  Cells with mixed degrees
   (degree-class boundaries) fall back to one-hot routing: node-major
   pretransform (stream tile as the stationary operand) plus a DVE-built
   one-hot matmul.  The cheaper budget wins per cell.
 - For each core the host materializes the source feature stream (raw
   input features, feature-major bf16, pre-scaled by 1/deg(dst) and
   shifted by bp @ Wp^-1 so the pretransform bias survives), read strictly
   sequentially by large DMAs.  No gathers, no dynamic descriptors.
 - Epilogue per 4 blocks (feature-major, alpha/hetero weights folded):
       mid = relu(w1^T x_t + ws^T aggS + wd^T aggD + wc^T aggC + bmid)
       out = wo^T mid + bout
Math (alpha = 0.5, folded on host):
  w1 = 0.5 W_self + 0.5 W_ct_r + I,  ws = 0.25 W_s2d, wd = 0.25 W_d2s,
  wc = 0.5 W_ct_l, bmid = 0.5 b_self + 0.25 b_s2d + 0.25 b_d2s + 0.5 b_ct_l
"""
import sys
import dataclasses

sys.path.insert(0, "/opt/trn_rl_repo")

import numpy as np
import ml_dtypes

import concourse.bass as bass
import concourse.bacc as bacc
import concourse.mybir as mybir
import concourse.tile as tile
from concourse import bass_utils

BF16 = ml_dtypes.bfloat16
F32 = np.float32
NCORE = 8
P = 128


@dataclasses.dataclass(frozen=True)
class Cfg:
    n_t: int = 100000
    n_c: int = 20000
    shard: int = 12800       # target nodes per core (multiple of 128)
    chunk_t: int = 128       # stream tiles per DMA chunk (128 tiles = 4 MB)
    run: int = 4             # max tiles per pretransform batch (1 PSUM bank)
    ogrp: int = 8            # output blocks per DMA
    egrp: int = 4            # epilogue batch (blocks)

    @property
    def nt_pad(self):
        return self.shard * NCORE

    @property
    def nblk(self):
        return self.shard // P


FULL = Cfg()

_prog_cache = {}


def _tiles_of(budgets, modes):
    """Flatten per-(block, dir) budgets into the static tile schedule."""
    Bs, Bd, Bc = budgets
    Ms, Md, Mc = modes
    tiles = []  # (blk, reg, is_ct, is_id, reg_first, reg_last, blk_last)
    for blk in range(len(Bs)):
        ccs = [Bs[blk] // P, Bd[blk] // P, Bc[blk] // P]
        ids = [Ms[blk], Md[blk], Mc[blk]]
        tot = sum(ccs)
        seen = 0
        for reg, cc in enumerate(ccs):
            for j in range(cc):
                seen += 1
                tiles.append((blk, reg, reg == 2, bool(ids[reg]), j == 0,
                              j == cc - 1, seen == tot))
    return tiles


def _chunks_of(T, cfg):
    """DMA chunk schedule: a few small leading chunks so compute starts
    early, then full-size chunks."""
    sched = []
    t = 0
    for n in (16, 16, 32, 64):
        if t >= T:
            break
        n = min(n, T - t)
        sched.append((t, n))
        t += n
    while t < T:
        n = min(cfg.chunk_t, T - t)
        sched.append((t, n))
        t += n
    return sched


_V7_COMPAT = True  # plain 128-tile chunks, no pre-issue (measured fastest)


def _chunks_plain(T, cfg):
    sched = []
    t = 0
    while t < T:
        n = min(cfg.chunk_t, T - t)
        sched.append((t, n))
        t += n
    return sched


def _groups_of(tiles, cfg, chunk_starts):
    """Batches of consecutive tiles: same mode, same weight (for identity
    runs), <= cfg.run tiles, never crossing a DMA chunk boundary."""
    groups = []  # (t0, n)
    t = 0
    while t < len(tiles):
        is_ct, is_id = tiles[t][2], tiles[t][3]
        n = 1
        while (n < cfg.run and t + n < len(tiles)
               and tiles[t + n][3] == is_id
               and (not is_id or tiles[t + n][2] == is_ct)
               and (t + n) not in chunk_starts):
            n += 1
        groups.append((t, n))
        t += n
    return groups


def build_program(cfg: Cfg, key):
    budgets, modes = key[:3], key[3:]
    Bs, Bd, Bc = budgets
    nblk = cfg.nblk
    tiles = _tiles_of(budgets, modes)
    chunks = (_chunks_plain if _V7_COMPAT else _chunks_of)(len(tiles), cfg)
    chunk_starts = {t0: (t0, n) for (t0, n) in chunks}
    groups = _groups_of(tiles, cfg, chunk_starts)
    T = len(tiles)
    S = T * P
    dt = mybir.dt
    AF = mybir.ActivationFunctionType
    OP = mybir.AluOpType

    nc = bacc.Bacc("TRN2", target_bir_lowering=False, debug=False)

    def din(name, shape, d):
        return nc.dram_tensor(name, shape, d, kind="ExternalInput")

    t_xTm = din("xTm", [P, cfg.shard], dt.bfloat16)
    t_stream = din("stream", [P, S], dt.bfloat16)
    t_dl = din("dl", [P, max(T, 1)], dt.bfloat16)
    t_wpt = din("wpt", [P, P], dt.bfloat16)
    t_wpc = din("wpc", [P, P], dt.bfloat16)
    t_bpt = din("bpt", [P, 1], dt.float32)
    t_w1 = din("w1", [P, P], dt.bfloat16)
    t_ws = din("ws", [P, P], dt.bfloat16)
    t_wd = din("wd", [P, P], dt.bfloat16)
    t_wc = din("wc", [P, P], dt.bfloat16)
    t_wo = din("wo", [P, P], dt.bfloat16)
    t_bmid = din("bmid", [P, 1], dt.float32)
    t_bout = din("bout", [P, 1], dt.float32)
    t_ident = din("ident", [P, P], dt.bfloat16)
    t_iota = din("iota", [P, P], dt.bfloat16)
    t_out = nc.dram_tensor("outT", [P, cfg.shard], dt.bfloat16, kind="ExternalOutput")

    with tile.TileContext(nc) as tc:
        with tc.tile_pool(name="persist", bufs=1) as pp, \
             tc.tile_pool(name="ch", bufs=3) as chp, \
             tc.tile_pool(name="rr", bufs=3) as rrp, \
             tc.tile_pool(name="ohp", bufs=3) as ohp, \
             tc.tile_pool(name="agp", bufs=2) as agp, \
             tc.tile_pool(name="mip", bufs=2) as mip, \
             tc.tile_pool(name="ogp", bufs=2) as ogp, \
             tc.tile_pool(name="psP", bufs=3, space="PSUM") as psP, \
             tc.tile_pool(name="psA", bufs=2, space="PSUM") as psA, \
             tc.tile_pool(name="psM", bufs=1, space="PSUM") as psM:
            def load(t, shape, d):
                s = pp.tile(shape, d, name=f"sb_{t.name}")
                nc.sync.dma_start(s[:], t.ap())
                return s

            sb_wpt = load(t_wpt, [P, P], dt.bfloat16)
            sb_wpc = load(t_wpc, [P, P], dt.bfloat16)
            sb_bpt = load(t_bpt, [P, 1], dt.float32)
            sb_w1 = load(t_w1, [P, P], dt.bfloat16)
            sb_ws = load(t_ws, [P, P], dt.bfloat16)
            sb_wd = load(t_wd, [P, P], dt.bfloat16)
            sb_wc = load(t_wc, [P, P], dt.bfloat16)
            sb_wo = load(t_wo, [P, P], dt.bfloat16)
            sb_bmid = load(t_bmid, [P, 1], dt.float32)
            sb_bout = load(t_bout, [P, 1], dt.float32)
            sb_ident = load(t_ident, [P, P], dt.bfloat16)
            sb_iota = load(t_iota, [P, P], dt.bfloat16)
            sb_dl = load(t_dl, [P, max(T, 1)], dt.bfloat16)

            # pre-issue the leading stream chunks so compute starts early
            chunk_tiles = {}
            if not _V7_COMPAT:
                for ci, (ct0, cn) in enumerate(chunks[:4]):
                    cs = chp.tile([P, cn * P], dt.bfloat16,
                                  name="chunk", tag=f"chunkp{ci}")
                    nc.sync.dma_start(cs[:],
                                      t_stream.ap()[:, ct0 * P:(ct0 + cn) * P])
                    chunk_tiles[ct0] = cs

            sb_xTm = load(t_xTm, [P, cfg.shard], dt.bfloat16)
            xt_sb = pp.tile([P, cfg.shard], dt.bfloat16)

            # ---- own-shard pretransform (feature-major, stationary Wp_t) ----
            for st in range(0, cfg.shard, 512):
                ps = psP.tile([P, cfg.run * P], dt.float32, name="pre", tag="pre")
                nc.tensor.matmul(ps[:], lhsT=sb_wpt[:],
                                 rhs=sb_xTm[:, st:st + 512],
                                 start=True, stop=True)
                nc.scalar.activation(xt_sb[:, st:st + 512], ps[:],
                                     AF.Relu, bias=sb_bpt[:, 0:1])

            # ---- main stream loop ----
            if True:

                chunk_sb = None
                agg_ps = [None]
                og = [None]
                eng_i = [0]
                next_epi = [0]
                epi_q = []
                EG = cfg.egrp

                def flush_epi():
                    if not epi_q:
                        return
                    q = epi_q[:]
                    epi_q.clear()
                    g = len(q)
                    b0 = q[0][0]
                    ps_mo = psM.tile([P, 2 * EG * P], dt.float32,
                                     name="mo", tag="mo")
                    ps_mid = ps_mo[:, 0:EG * P]
                    ps_out = ps_mo[:, EG * P:2 * EG * P]
                    nc.tensor.matmul(ps_mid[:, 0:g * P], lhsT=sb_w1[:],
                                     rhs=xt_sb[:, b0 * P:(b0 + g) * P],
                                     start=True, stop=False)
                    for i, (blk, sb_agg) in enumerate(q):
                        terms = []
                        if Bs[blk]:
                            terms.append((sb_ws, sb_agg[:, 0:P]))
                        if Bd[blk]:
                            terms.append((sb_wd, sb_agg[:, P:2 * P]))
                        if Bc[blk]:
                            terms.append((sb_wc, sb_agg[:, 2 * P:3 * P]))
                        for k, (wsb, rhs) in enumerate(terms):
                            nc.tensor.matmul(ps_mid[:, i * P:(i + 1) * P],
                                             lhsT=wsb[:], rhs=rhs, start=False,
                                             stop=(k == len(terms) - 1),
                                             skip_group_check=True)
                    sb_mid = mip.tile([P, EG * P], dt.bfloat16,
                                      name="smid", tag="smid")
                    nc.scalar.activation(sb_mid[:, 0:g * P], ps_mid[:, 0:g * P],
                                         AF.Relu, bias=sb_bmid[:, 0:1])
                    nc.tensor.matmul(ps_out[:, 0:g * P], lhsT=sb_wo[:],
                                     rhs=sb_mid[:, 0:g * P], start=True, stop=True)
                    if b0 % cfg.ogrp == 0:
                        og[0] = ogp.tile([P, cfg.ogrp * P], dt.bfloat16,
                                         name="og", tag="og")
                    oo = (b0 % cfg.ogrp) * P
                    nc.scalar.activation(og[0][:, oo:oo + g * P],
                                         ps_out[:, 0:g * P],
                                         AF.Identity, bias=sb_bout[:, 0:1])
                    bl = b0 + g - 1
                    if bl % cfg.ogrp == cfg.ogrp - 1 or bl == nblk - 1:
                        g0 = (bl // cfg.ogrp) * cfg.ogrp
                        gn = bl - g0 + 1
                        nc.sync.dma_start(t_out.ap()[:, g0 * P:(g0 + gn) * P],
                                          og[0][:, :gn * P])

                def enqueue_epi(blk, sb_agg):
                    epi_q.append((blk, sb_agg))
                    if len(epi_q) == EG:
                        flush_epi()

                def finish_block(blk):
                    used = [(0, Bs[blk]), (1, Bd[blk]), (2, Bc[blk])]
                    sb_agg = agp.tile([P, 3 * P], dt.bfloat16,
                                      name="sagg", tag="sagg")
                    runs_ = []
                    for reg, B in used:
                        if not B:
                            continue
                        if runs_ and runs_[-1][1] == reg:
                            runs_[-1] = (runs_[-1][0], reg + 1)
                        else:
                            runs_.append((reg, reg + 1))
                    for a, b in runs_:
                        if blk % 2 == 0:
                            nc.vector.tensor_copy(sb_agg[:, a * P:b * P],
                                                  agg_ps[0][:, a * P:b * P])
                        else:
                            nc.scalar.copy(sb_agg[:, a * P:b * P],
                                           agg_ps[0][:, a * P:b * P])
                    while next_epi[0] < blk:
                        enqueue_epi(next_epi[0], sb_agg)
                        next_epi[0] += 1
                    enqueue_epi(blk, sb_agg)
                    next_epi[0] = blk + 1

                def seg_mm(i, r_ap, oh):
                    blk, reg, _, is_id, first, last, blk_last = tiles[i]
                    if agg_ps[0] is None:
                        agg_ps[0] = psA.tile([P, 3 * P], dt.float32,
                                             name="agg", tag="agg")
                    if is_id:
                        nc.tensor.matmul(agg_ps[0][:, reg * P:(reg + 1) * P],
                                         lhsT=sb_ident[:], rhs=r_ap,
                                         start=first, stop=last)
                    else:
                        nc.tensor.matmul(agg_ps[0][:, reg * P:(reg + 1) * P],
                                         lhsT=r_ap, rhs=oh,
                                         start=first, stop=last)
                    if blk_last:
                        finish_block(blk)
                        agg_ps[0] = None

                cur_c0 = -1
                for (t0, n) in groups:
                    if t0 in chunk_starts:
                        ct0, cn = chunk_starts[t0]
                        if ct0 in chunk_tiles:
                            chunk_sb = chunk_tiles[ct0]
                        else:
                            chunk_sb = chp.tile([P, cfg.chunk_t * P],
                                                dt.bfloat16,
                                                name="chunk", tag="chunk")
                            nc.sync.dma_start(
                                chunk_sb[:, :cn * P],
                                t_stream.ap()[:, ct0 * P:(ct0 + cn) * P])
                        cur_c0 = ct0

                    is_id = tiles[t0][3]
                    off = (t0 - cur_c0) * P
                    ps_run = psP.tile([P, cfg.run * P], dt.float32,
                                      name="pre", tag="pre")
                    if is_id:
                        # feature-major pretransform, stationary Wp
                        is_ct = tiles[t0][2]
                        nc.tensor.matmul(ps_run[:, 0:n * P],
                                         lhsT=(sb_wpc if is_ct else sb_wpt)[:],
                                         rhs=chunk_sb[:, off:off + n * P],
                                         start=True, stop=True)
                    else:
                        # node-major pretransform, stream tiles stationary
                        for i in range(n):
                            nc.tensor.matmul(
                                ps_run[:, i * P:(i + 1) * P],
                                lhsT=chunk_sb[:, off + i * P:off + (i + 1) * P],
                                rhs=(sb_wpc if tiles[t0 + i][2] else sb_wpt)[:],
                                start=True, stop=True)
                    r_sb = rrp.tile([P, cfg.run * P], dt.bfloat16,
                                    name="r", tag="r")
                    if eng_i[0] % 9 < 4:
                        nc.vector.tensor_scalar_max(r_sb[:, 0:n * P],
                                                    ps_run[:, 0:n * P], 0.0)
                    else:
                        nc.scalar.activation(r_sb[:, 0:n * P],
                                             ps_run[:, 0:n * P], AF.Relu)
                    eng_i[0] += 1

                    oh = None
                    if not is_id:
                        oh = ohp.tile([P, cfg.run, P], dt.bfloat16,
                                      name="oh", tag="oh")
                        nc.vector.tensor_tensor(
                            out=oh[:, :n, :],
                            in0=sb_iota[:].unsqueeze(1).to_broadcast([P, n, P]),
                            in1=sb_dl[:, t0:t0 + n].unsqueeze(2)
                                .to_broadcast([P, n, P]),
                            op=OP.is_equal)

                    for i in range(n):
                        seg_mm(t0 + i, r_sb[:, i * P:(i + 1) * P],
                               oh[:, i, :] if oh is not None else None)

                while next_epi[0] < nblk:
                    enqueue_epi(next_epi[0], None)
                    next_epi[0] += 1
                flush_epi()

    nc.compile()
    return nc


def _solve_shift(W, b):
    """delta s.t. delta @ W == b (for folding the pretransform bias into x)."""
    if not np.any(b):
        return np.zeros_like(b)
    try:
        d = np.linalg.solve(W.T.astype(np.float64), b.astype(np.float64))
    except np.linalg.LinAlgError:
        d = np.linalg.lstsq(W.T.astype(np.float64), b.astype(np.float64),
                            rcond=None)[0]
    assert np.allclose(d @ W.astype(np.float64), b, atol=1e-4), \
        "pretransform weight not invertible; bias fold failed"
    return d.astype(F32)


def preprocess(inputs, cfg: Cfg):
    xt = np.asarray(inputs["x_target"], F32)
    xc = np.asarray(inputs["x_context"], F32)
    ett = np.asarray(inputs["edge_tt"]).astype(np.int64)
    ecs = np.asarray(inputs["edge_ct_src"]).astype(np.int64)
    ecd = np.asarray(inputs["edge_ct_dst"]).astype(np.int64)
    n_t = xt.shape[0]
    nblk = cfg.nblk
    nb = NCORE * nblk

    Wp_t = np.asarray(inputs["Wp_t"], F32)
    Wp_c = np.asarray(inputs["Wp_c"], F32)
    bp_t = np.asarray(inputs["bp_t"], F32)
    bp_c = np.asarray(inputs["bp_c"], F32)

    xtT = (xt + _solve_shift(Wp_t, bp_t)).T.copy()
    xcT = (xc + _solve_shift(Wp_c, bp_c)).T.copy()

    W_self = np.asarray(inputs["W_self"], F32)
    W_ct_r = np.asarray(inputs["W_ct_r"], F32)
    w1 = 0.5 * W_self + 0.5 * W_ct_r + np.eye(P, dtype=F32)
    ws = 0.25 * np.asarray(inputs["W_s2d"], F32)
    wd = 0.25 * np.asarray(inputs["W_d2s"], F32)
    wc = 0.5 * np.asarray(inputs["W_ct_l"], F32)
    wo = np.asarray(inputs["W_out"], F32)
    bmid = (0.5 * np.asarray(inputs["b_self"], F32)
            + 0.25 * np.asarray(inputs["b_s2d"], F32)
            + 0.25 * np.asarray(inputs["b_d2s"], F32)
            + 0.5 * np.asarray(inputs["b_ct_l"], F32))
    bout = np.asarray(inputs["b_out"], F32)

    shared = {
        "wpt": np.ascontiguousarray(Wp_t.astype(BF16)),
        "wpc": np.ascontiguousarray(Wp_c.astype(BF16)),
        "bpt": bp_t.reshape(P, 1),
        "w1": w1.astype(BF16), "ws": ws.astype(BF16), "wd": wd.astype(BF16),
        "wc": wc.astype(BF16), "wo": wo.astype(BF16),
        "bmid": bmid.reshape(P, 1), "bout": bout.reshape(P, 1),
        "ident": np.eye(P, dtype=F32).astype(BF16),
        "iota": np.ascontiguousarray(
            np.broadcast_to(np.arange(P, dtype=F32), (P, P)).astype(BF16)),
    }

    # degree-grouped relabeling (see module docstring)
    deg = {
        "s": np.bincount(ett[1], minlength=n_t),
        "d": np.bincount(ett[0], minlength=n_t),
        "c": np.bincount(ecd, minlength=n_t),
    }
    # within (s,d)-degree classes use a deterministic random order so each
    # block samples the ct-degree distribution evenly (ct always one-hot)
    rnd = np.random.RandomState(0).permutation(n_t)
    norder = np.lexsort((rnd, deg["d"], deg["s"]))
    pos = np.arange(n_t)
    node_gblk = np.empty(n_t, np.int64)
    node_loc = np.empty(n_t, np.int64)
    node_gblk[norder] = pos // P
    node_loc[norder] = pos % P

    # per-global-block stats, then group 8 blocks with similar stats into
    # each (slot x 8 cores) so per-slot maxima stay tight in every direction
    stats = {}
    for nm in ("s", "d", "c"):
        dsort = np.zeros(nb * P, np.int64)
        dsort[:n_t] = deg[nm][norder]
        per_blk = dsort.reshape(nb, P)
        stats[nm] = (per_blk.max(axis=1), per_blk.sum(axis=1))
    bsort = np.lexsort((stats["c"][1], stats["c"][0],
                        stats["d"][1], stats["d"][0],
                        stats["s"][1], stats["s"][0]))
    q = np.arange(nb)
    core_of = np.empty(nb, np.int64)
    slot_of = np.empty(nb, np.int64)
    core_of[bsort] = q % NCORE
    slot_of[bsort] = q // NCORE
    node_core = core_of[node_gblk]
    node_slot = slot_of[node_gblk]
    node_col = node_core * cfg.shard + node_slot * P + node_loc

    # per-(slot, dir): identity budget (128*maxdeg) vs one-hot budget
    # (ceil128 of max cell count); the cheaper mode wins.
    budgets, modes = {}, {}
    for nm in ("s", "d", "c"):
        md, ct = stats[nm]
        blkmax = md[bsort].reshape(nblk, NCORE).max(axis=1)
        blkcnt = ct[bsort].reshape(nblk, NCORE).max(axis=1)
        B_id = blkmax * P
        B_oh = ((blkcnt + P - 1) // P) * P
        use_id = B_id <= B_oh + 2 * P
        budgets[nm] = np.where(use_id, B_id, B_oh)
        modes[nm] = use_id
    Bs, Bd, Bc = budgets["s"], budgets["d"], budgets["c"]
    Ms, Md, Mc = modes["s"], modes["d"], modes["c"]

    off = np.zeros(nblk, np.int64)
    acc = 0
    for blk in range(nblk):
        off[blk] = acc
        acc += Bs[blk] + Bd[blk] + Bc[blk]
    S = int(acc)
    T = S // P
    reg_off = {"s": np.zeros(nblk, np.int64), "d": Bs.copy(),
               "c": (Bs + Bd).copy()}

    dirs = {
        "s": (ett[1], ett[0], xtT),
        "d": (ett[0], ett[1], xtT),
        "c": (ecd, ecs, xcT),
    }

    in_maps = [dict(shared) for _ in range(NCORE)]
    xrawT = np.zeros((P, cfg.nt_pad), BF16)
    xrawT[:, node_col] = xt.T.astype(BF16)
    for k in range(NCORE):
        in_maps[k]["xTm"] = np.ascontiguousarray(
            xrawT[:, k * cfg.shard:(k + 1) * cfg.shard])

    # per-edge placement
    place = {}
    for nm, (key, gnode, srcT) in dirs.items():
        use_id = modes[nm]
        order = np.argsort(key, kind="stable")
        key_s = key[order]
        starts = np.concatenate(
            [[0], np.cumsum(np.bincount(key_s, minlength=n_t))[:-1]])
        rank = np.arange(len(key_s)) - starts[key_s]
        v = (1.0 / np.maximum(deg[nm], 1))[key_s].astype(F32)
        slot_blk = node_slot[key_s]
        loc = node_loc[key_s]
        # identity cells: slot by (rank, dst local)
        slot = rank * P + loc
        # one-hot cells: slot by arrival order within the (core, slot) cell
        ohsel = ~use_id[slot_blk]
        if ohsel.any():
            cell = (node_core[key_s] * nblk + slot_blk)
            cord = np.argsort(cell[ohsel], kind="stable")
            cell_o = cell[ohsel][cord]
            cstarts = np.concatenate(
                [[0], np.cumsum(np.bincount(cell_o, minlength=NCORE * nblk))[:-1]])
            cpos = np.arange(len(cell_o)) - cstarts[cell_o]
            tmp = np.empty(ohsel.sum(), np.int64)
            tmp[cord] = cpos
            slot[ohsel] = tmp
        slot = off[slot_blk] + reg_off[nm][slot_blk] + slot
        place[nm] = (order, key_s, slot, v, gnode, loc)

    for k in range(NCORE):
        stream = np.zeros((P, S), F32)
        dlf = np.full(S, -1.0, F32)
        for nm, (order, key_s, slot, v, gnode, loc) in place.items():
            sel = node_core[key_s] == k
            o = order[sel]
            stream[:, slot[sel]] = dirs[nm][2][:, gnode[o]] * v[sel][None, :]
            dlf[slot[sel]] = loc[sel]
        in_maps[k]["stream"] = stream.astype(BF16)
        in_maps[k]["dl"] = np.ascontiguousarray(
            dlf.reshape(T, P).T.astype(BF16))

    bkey = (tuple(Bs.tolist()), tuple(Bd.tolist()), tuple(Bc.tolist()),
            tuple(bool(x) for x in Ms), tuple(bool(x) for x in Md),
            tuple(bool(x) for x in Mc))
    return in_maps, bkey, node_col


def run(inputs, cfg: Cfg, trace=False, tmpdir=None, trace_cores=None):
    in_maps, bkey, node_col = preprocess(inputs, cfg)
    if bkey not in _prog_cache:
        _prog_cache[bkey] = build_program(cfg, bkey)
    nc = _prog_cache[bkey]
    res = bass_utils.run_bass_kernel_spmd(nc, in_maps, core_ids=list(range(NCORE)),
                                          trace=trace, tmpdir=tmpdir,
                                          trace_cores=trace_cores)
    outT = np.concatenate([res.results[k]["outT"] for k in range(NCORE)], axis=1)
    out = outT[:, node_col].T.astype(F32)
    return out, res


def kernel(**inputs) -> np.ndarray:
    out, _ = run(inputs, FULL, trace=False)
    return out


# revision 54
# speedup vs baseline: 1.0129x; 1.0129x over previous
"""Trainium2 Bass kernel for nn_HeteroForecastSageConv.

Strategy (8 NeuronCores, SPMD, degree-grouped edge streams):
 - Target (destination) nodes are relabeled by lexsorting on their
   (s2d, d2s, ct) in-degrees and chunking 128 consecutive nodes per block;
   blocks are dealt round-robin over the 8 cores.  Within most blocks the
   per-direction degree is constant, so that (block, direction) cell lays
   its edges out as full 128-wide tiles where slot j of tile r holds the
   r-th edge of destination j: the routing matrix is the identity and the
   segment sum is a plain PSUM accumulation of feature-major tiles
   (pretransform with stationary Wp at N<=512).  Cells with mixed degrees
   (degree-class boundaries) fall back to one-hot routing: node-major
   pretransform (stream tile as the stationary operand) plus a DVE-built
   one-hot matmul.  The cheaper budget wins per cell.
 - For each core the host materializes the source feature stream (raw
   input features, feature-major bf16, pre-scaled by 1/deg(dst) and
   shifted by bp @ Wp^-1 so the pretransform bias survives), read strictly
   sequentially by large DMAs.  No gathers, no dynamic descriptors.
 - Epilogue per 4 blocks (feature-major, alpha/hetero weights folded):
       mid = relu(w1^T x_t + ws^T aggS + wd^T aggD + wc^T aggC + bmid)
       out = wo^T mid + bout
Math (alpha = 0.5, folded on host):
  w1 = 0.5 W_self + 0.5 W_ct_r + I,  ws = 0.25 W_s2d, wd = 0.25 W_d2s,
  wc = 0.5 W_ct_l, bmid = 0.5 b_self + 0.25 b_s2d + 0.25 b_d2s + 0.5 b_ct_l
"""
import sys
import dataclasses

sys.path.insert(0, "/opt/trn_rl_repo")

import numpy as np
import ml_dtypes

import concourse.bass as bass
import concourse.bacc as bacc
import concourse.mybir as mybir
import concourse.tile as tile
from concourse import bass_utils

BF16 = ml_dtypes.bfloat16
F32 = np.float32
NCORE = 8
P = 128


@dataclasses.dataclass(frozen=True)
class Cfg:
    n_t: int = 100000
    n_c: int = 20000
    shard: int = 12800       # target nodes per core (multiple of 128)
    chunk_t: int = 128       # stream tiles per DMA chunk (128 tiles = 4 MB)
    run: int = 4             # max tiles per pretransform batch (1 PSUM bank)
    ogrp: int = 8            # output blocks per DMA
    egrp: int = 4            # epilogue batch (blocks)

    @property
    def nt_pad(self):
        return self.shard * NCORE

    @property
    def nblk(self):
        return self.shard // P


FULL = Cfg()

_prog_cache = {}


def _tiles_of(budgets, modes):
    """Flatten per-(block, dir) budgets into the static tile schedule."""
    Bs, Bd, Bc = budgets
    Ms, Md, Mc = modes
    tiles = []  # (blk, reg, is_ct, is_id, reg_first, reg_last, blk_last)
    for blk in range(len(Bs)):
        ccs = [Bs[blk] // P, Bd[blk] // P, Bc[blk] // P]
        ids = [Ms[blk], Md[blk], Mc[blk]]
        tot = sum(ccs)
        seen = 0
        for reg, cc in enumerate(ccs):
            for j in range(cc):
                seen += 1
                tiles.append((blk, reg, reg == 2, bool(ids[reg]), j == 0,
                              j == cc - 1, seen == tot))
    return tiles


def _chunks_of(T, cfg):
    """DMA chunk schedule: a few small leading chunks so compute starts
    early, then full-size chunks."""
    sched = []
    t = 0
    for n in (16, 16, 32, 64):
        if t >= T:
            break
        n = min(n, T - t)
        sched.append((t, n))
        t += n
    while t < T:
        n = min(cfg.chunk_t, T - t)
        sched.append((t, n))
        t += n
    return sched


_V7_COMPAT = True  # plain 128-tile chunks, no pre-issue (measured fastest)


def _chunks_plain(T, cfg):
    sched = []
    t = 0
    while t < T:
        n = min(cfg.chunk_t, T - t)
        sched.append((t, n))
        t += n
    return sched


def _groups_of(tiles, cfg, chunk_starts):
    """Batches of consecutive tiles: same mode, same weight (for identity
    runs), <= cfg.run tiles, never crossing a DMA chunk boundary."""
    groups = []  # (t0, n)
    t = 0
    while t < len(tiles):
        is_ct, is_id = tiles[t][2], tiles[t][3]
        n = 1
        while (n < cfg.run and t + n < len(tiles)
               and tiles[t + n][3] == is_id
               and (not is_id or tiles[t + n][2] == is_ct)
               and (t + n) not in chunk_starts):
            n += 1
        groups.append((t, n))
        t += n
    return groups


def build_program(cfg: Cfg, key):
    budgets, modes = key[:3], key[3:]
    Bs, Bd, Bc = budgets
    nblk = cfg.nblk
    tiles = _tiles_of(budgets, modes)
    chunks = (_chunks_plain if _V7_COMPAT else _chunks_of)(len(tiles), cfg)
    chunk_starts = {t0: (t0, n) for (t0, n) in chunks}
    groups = _groups_of(tiles, cfg, chunk_starts)
    T = len(tiles)
    S = T * P
    dt = mybir.dt
    AF = mybir.ActivationFunctionType
    OP = mybir.AluOpType

    nc = bacc.Bacc("TRN2", target_bir_lowering=False, debug=False)

    def din(name, shape, d):
        return nc.dram_tensor(name, shape, d, kind="ExternalInput")

    t_xTm = din("xTm", [P, cfg.shard], dt.bfloat16)
    t_stream = din("stream", [P, S], dt.bfloat16)
    t_dl = din("dl", [P, max(T, 1)], dt.bfloat16)
    t_wpt = din("wpt", [P, P], dt.bfloat16)
    t_wpc = din("wpc", [P, P], dt.bfloat16)
    t_bpt = din("bpt", [P, 1], dt.float32)
    t_w1 = din("w1", [P, P], dt.bfloat16)
    t_ws = din("ws", [P, P], dt.bfloat16)
    t_wd = din("wd", [P, P], dt.bfloat16)
    t_wc = din("wc", [P, P], dt.bfloat16)
    t_wo = din("wo", [P, P], dt.bfloat16)
    t_bmid = din("bmid", [P, 1], dt.float32)
    t_bout = din("bout", [P, 1], dt.float32)
    t_ident = din("ident", [P, P], dt.bfloat16)
    t_iota = din("iota", [P, P], dt.bfloat16)
    t_out = nc.dram_tensor("outT", [P, cfg.shard], dt.bfloat16, kind="ExternalOutput")

    with tile.TileContext(nc) as tc:
        with tc.tile_pool(name="persist", bufs=1) as pp, \
             tc.tile_pool(name="ch", bufs=2) as chp, \
             tc.tile_pool(name="rr", bufs=3) as rrp, \
             tc.tile_pool(name="ohp", bufs=3) as ohp, \
             tc.tile_pool(name="agp", bufs=2) as agp, \
             tc.tile_pool(name="mip", bufs=2) as mip, \
             tc.tile_pool(name="ogp", bufs=2) as ogp, \
             tc.tile_pool(name="psP", bufs=3, space="PSUM") as psP, \
             tc.tile_pool(name="psA", bufs=2, space="PSUM") as psA, \
             tc.tile_pool(name="psM", bufs=1, space="PSUM") as psM:
            def load(t, shape, d):
                s = pp.tile(shape, d, name=f"sb_{t.name}")
                nc.sync.dma_start(s[:], t.ap())
                return s

            sb_wpt = load(t_wpt, [P, P], dt.bfloat16)
            sb_wpc = load(t_wpc, [P, P], dt.bfloat16)
            sb_bpt = load(t_bpt, [P, 1], dt.float32)
            sb_w1 = load(t_w1, [P, P], dt.bfloat16)
            sb_ws = load(t_ws, [P, P], dt.bfloat16)
            sb_wd = load(t_wd, [P, P], dt.bfloat16)
            sb_wc = load(t_wc, [P, P], dt.bfloat16)
            sb_wo = load(t_wo, [P, P], dt.bfloat16)
            sb_bmid = load(t_bmid, [P, 1], dt.float32)
            sb_bout = load(t_bout, [P, 1], dt.float32)
            sb_ident = load(t_ident, [P, P], dt.bfloat16)
            sb_iota = load(t_iota, [P, P], dt.bfloat16)

            # pre-issue the first stream chunk ahead of the big persistent
            # loads so the PE starts on stream tiles ~10us earlier
            chunk_tiles = {}
            pre_n = 1 if _V7_COMPAT else 4
            for ci, (ct0, cn) in enumerate(chunks[:pre_n]):
                tag = "chunk" if _V7_COMPAT else f"chunkp{ci}"
                cs = chp.tile([P, cn * P], dt.bfloat16, name="chunk", tag=tag)
                nc.sync.dma_start(cs[:],
                                  t_stream.ap()[:, ct0 * P:(ct0 + cn) * P])
                chunk_tiles[ct0] = cs

            sb_dl = load(t_dl, [P, max(T, 1)], dt.bfloat16)
            sb_xTm = load(t_xTm, [P, cfg.shard], dt.bfloat16)
            xt_sb = pp.tile([P, cfg.shard], dt.bfloat16)

            # ---- own-shard pretransform (feature-major, stationary Wp_t) ----
            for st in range(0, cfg.shard, 512):
                ps = psP.tile([P, cfg.run * P], dt.float32, name="pre", tag="pre")
                nc.tensor.matmul(ps[:], lhsT=sb_wpt[:],
                                 rhs=sb_xTm[:, st:st + 512],
                                 start=True, stop=True)
                nc.scalar.activation(xt_sb[:, st:st + 512], ps[:],
                                     AF.Relu, bias=sb_bpt[:, 0:1])

            # ---- main stream loop ----
            if True:

                chunk_sb = None
                agg_ps = [None]
                og = [None]
                eng_i = [0]
                next_epi = [0]
                epi_q = []
                EG = cfg.egrp

                def flush_epi():
                    if not epi_q:
                        return
                    q = epi_q[:]
                    epi_q.clear()
                    g = len(q)
                    b0 = q[0][0]
                    ps_mo = psM.tile([P, 2 * EG * P], dt.float32,
                                     name="mo", tag="mo")
                    ps_mid = ps_mo[:, 0:EG * P]
                    ps_out = ps_mo[:, EG * P:2 * EG * P]
                    nc.tensor.matmul(ps_mid[:, 0:g * P], lhsT=sb_w1[:],
                                     rhs=xt_sb[:, b0 * P:(b0 + g) * P],
                                     start=True, stop=False)
                    for i, (blk, sb_agg) in enumerate(q):
                        terms = []
                        if Bs[blk]:
                            terms.append((sb_ws, sb_agg[:, 0:P]))
                        if Bd[blk]:
                            terms.append((sb_wd, sb_agg[:, P:2 * P]))
                        if Bc[blk]:
                            terms.append((sb_wc, sb_agg[:, 2 * P:3 * P]))
                        for k, (wsb, rhs) in enumerate(terms):
                            nc.tensor.matmul(ps_mid[:, i * P:(i + 1) * P],
                                             lhsT=wsb[:], rhs=rhs, start=False,
                                             stop=(k == len(terms) - 1),
                                             skip_group_check=True)
                    sb_mid = mip.tile([P, EG * P], dt.bfloat16,
                                      name="smid", tag="smid")
                    nc.scalar.activation(sb_mid[:, 0:g * P], ps_mid[:, 0:g * P],
                                         AF.Relu, bias=sb_bmid[:, 0:1])
                    nc.tensor.matmul(ps_out[:, 0:g * P], lhsT=sb_wo[:],
                                     rhs=sb_mid[:, 0:g * P], start=True, stop=True)
                    if b0 % cfg.ogrp == 0:
                        og[0] = ogp.tile([P, cfg.ogrp * P], dt.bfloat16,
                                         name="og", tag="og")
                    oo = (b0 % cfg.ogrp) * P
                    nc.scalar.activation(og[0][:, oo:oo + g * P],
                                         ps_out[:, 0:g * P],
                                         AF.Identity, bias=sb_bout[:, 0:1])
                    bl = b0 + g - 1
                    if bl % cfg.ogrp == cfg.ogrp - 1 or bl == nblk - 1:
                        g0 = (bl // cfg.ogrp) * cfg.ogrp
                        gn = bl - g0 + 1
                        nc.sync.dma_start(t_out.ap()[:, g0 * P:(g0 + gn) * P],
                                          og[0][:, :gn * P])

                def enqueue_epi(blk, sb_agg):
                    epi_q.append((blk, sb_agg))
                    if len(epi_q) == EG:
                        flush_epi()

                def finish_block(blk):
                    used = [(0, Bs[blk]), (1, Bd[blk]), (2, Bc[blk])]
                    sb_agg = agp.tile([P, 3 * P], dt.bfloat16,
                                      name="sagg", tag="sagg")
                    runs_ = []
                    for reg, B in used:
                        if not B:
                            continue
                        if runs_ and runs_[-1][1] == reg:
                            runs_[-1] = (runs_[-1][0], reg + 1)
                        else:
                            runs_.append((reg, reg + 1))
                    for a, b in runs_:
                        nc.scalar.copy(sb_agg[:, a * P:b * P],
                                       agg_ps[0][:, a * P:b * P])
                    while next_epi[0] < blk:
                        enqueue_epi(next_epi[0], sb_agg)
                        next_epi[0] += 1
                    enqueue_epi(blk, sb_agg)
                    next_epi[0] = blk + 1

                def seg_mm(i, r_ap, oh):
                    blk, reg, _, is_id, first, last, blk_last = tiles[i]
                    if agg_ps[0] is None:
                        agg_ps[0] = psA.tile([P, 3 * P], dt.float32,
                                             name="agg", tag="agg")
                    if is_id:
                        nc.tensor.matmul(agg_ps[0][:, reg * P:(reg + 1) * P],
                                         lhsT=sb_ident[:], rhs=r_ap,
                                         start=first, stop=last)
                    else:
                        nc.tensor.matmul(agg_ps[0][:, reg * P:(reg + 1) * P],
                                         lhsT=r_ap, rhs=oh,
                                         start=first, stop=last)
                    if blk_last:
                        finish_block(blk)
                        agg_ps[0] = None

                cur_c0 = -1
                for (t0, n) in groups:
                    if t0 in chunk_starts:
                        ct0, cn = chunk_starts[t0]
                        if ct0 in chunk_tiles:
                            chunk_sb = chunk_tiles[ct0]
                        else:
                            chunk_sb = chp.tile([P, cfg.chunk_t * P],
                                                dt.bfloat16,
                                                name="chunk", tag="chunk")
                            nc.sync.dma_start(
                                chunk_sb[:, :cn * P],
                                t_stream.ap()[:, ct0 * P:(ct0 + cn) * P])
                        cur_c0 = ct0

                    is_id = tiles[t0][3]
                    off = (t0 - cur_c0) * P
                    ps_run = psP.tile([P, cfg.run * P], dt.float32,
                                      name="pre", tag="pre")
                    if is_id:
                        # feature-major pretransform, stationary Wp
                        is_ct = tiles[t0][2]
                        nc.tensor.matmul(ps_run[:, 0:n * P],
                                         lhsT=(sb_wpc if is_ct else sb_wpt)[:],
                                         rhs=chunk_sb[:, off:off + n * P],
                                         start=True, stop=True)
                    else:
                        # node-major pretransform, stream tiles stationary
                        for i in range(n):
                            nc.tensor.matmul(
                                ps_run[:, i * P:(i + 1) * P],
                                lhsT=chunk_sb[:, off + i * P:off + (i + 1) * P],
                                rhs=(sb_wpc if tiles[t0 + i][2] else sb_wpt)[:],
                                start=True, stop=True)
                    r_sb = rrp.tile([P, cfg.run * P], dt.bfloat16,
                                    name="r", tag="r")
                    if eng_i[0] % 2 == 0:
                        nc.vector.tensor_scalar_max(r_sb[:, 0:n * P],
                                                    ps_run[:, 0:n * P], 0.0)
                    else:
                        nc.scalar.activation(r_sb[:, 0:n * P],
                                             ps_run[:, 0:n * P], AF.Relu)
                    eng_i[0] += 1

                    oh = None
                    if not is_id:
                        oh = ohp.tile([P, cfg.run, P], dt.bfloat16,
                                      name="oh", tag="oh")
                        nc.vector.tensor_tensor(
                            out=oh[:, :n, :],
                            in0=sb_iota[:].unsqueeze(1).to_broadcast([P, n, P]),
                            in1=sb_dl[:, t0:t0 + n].unsqueeze(2)
                                .to_broadcast([P, n, P]),
                            op=OP.is_equal)

                    for i in range(n):
                        seg_mm(t0 + i, r_sb[:, i * P:(i + 1) * P],
                               oh[:, i, :] if oh is not None else None)

                while next_epi[0] < nblk:
                    enqueue_epi(next_epi[0], None)
                    next_epi[0] += 1
                flush_epi()

    nc.compile()
    return nc


def _solve_shift(W, b):
    """delta s.t. delta @ W == b (for folding the pretransform bias into x)."""
    if not np.any(b):
        return np.zeros_like(b)
    try:
        d = np.linalg.solve(W.T.astype(np.float64), b.astype(np.float64))
    except np.linalg.LinAlgError:
        d = np.linalg.lstsq(W.T.astype(np.float64), b.astype(np.float64),
                            rcond=None)[0]
    assert np.allclose(d @ W.astype(np.float64), b, atol=1e-4), \
        "pretransform weight not invertible; bias fold failed"
    return d.astype(F32)


def preprocess(inputs, cfg: Cfg):
    xt = np.asarray(inputs["x_target"], F32)
    xc = np.asarray(inputs["x_context"], F32)
    ett = np.asarray(inputs["edge_tt"]).astype(np.int64)
    ecs = np.asarray(inputs["edge_ct_src"]).astype(np.int64)
    ecd = np.asarray(inputs["edge_ct_dst"]).astype(np.int64)
    n_t = xt.shape[0]
    nblk = cfg.nblk
    nb = NCORE * nblk

    Wp_t = np.asarray(inputs["Wp_t"], F32)
    Wp_c = np.asarray(inputs["Wp_c"], F32)
    bp_t = np.asarray(inputs["bp_t"], F32)
    bp_c = np.asarray(inputs["bp_c"], F32)

    xtT = (xt + _solve_shift(Wp_t, bp_t)).T.copy()
    xcT = (xc + _solve_shift(Wp_c, bp_c)).T.copy()

    W_self = np.asarray(inputs["W_self"], F32)
    W_ct_r = np.asarray(inputs["W_ct_r"], F32)
    w1 = 0.5 * W_self + 0.5 * W_ct_r + np.eye(P, dtype=F32)
    ws = 0.25 * np.asarray(inputs["W_s2d"], F32)
    wd = 0.25 * np.asarray(inputs["W_d2s"], F32)
    wc = 0.5 * np.asarray(inputs["W_ct_l"], F32)
    wo = np.asarray(inputs["W_out"], F32)
    bmid = (0.5 * np.asarray(inputs["b_self"], F32)
            + 0.25 * np.asarray(inputs["b_s2d"], F32)
            + 0.25 * np.asarray(inputs["b_d2s"], F32)
            + 0.5 * np.asarray(inputs["b_ct_l"], F32))
    bout = np.asarray(inputs["b_out"], F32)

    shared = {
        "wpt": np.ascontiguousarray(Wp_t.astype(BF16)),
        "wpc": np.ascontiguousarray(Wp_c.astype(BF16)),
        "bpt": bp_t.reshape(P, 1),
        "w1": w1.astype(BF16), "ws": ws.astype(BF16), "wd": wd.astype(BF16),
        "wc": wc.astype(BF16), "wo": wo.astype(BF16),
        "bmid": bmid.reshape(P, 1), "bout": bout.reshape(P, 1),
        "ident": np.eye(P, dtype=F32).astype(BF16),
        "iota": np.ascontiguousarray(
            np.broadcast_to(np.arange(P, dtype=F32), (P, P)).astype(BF16)),
    }

    # degree-grouped relabeling (see module docstring)
    deg = {
        "s": np.bincount(ett[1], minlength=n_t),
        "d": np.bincount(ett[0], minlength=n_t),
        "c": np.bincount(ecd, minlength=n_t),
    }
    # within (s,d)-degree classes use a deterministic random order so each
    # block samples the ct-degree distribution evenly (ct always one-hot)
    rnd = np.random.RandomState(0).permutation(n_t)
    norder = np.lexsort((rnd, deg["d"], deg["s"]))
    pos = np.arange(n_t)
    node_gblk = np.empty(n_t, np.int64)
    node_loc = np.empty(n_t, np.int64)
    node_gblk[norder] = pos // P
    node_loc[norder] = pos % P

    # per-global-block stats, then group 8 blocks with similar stats into
    # each (slot x 8 cores) so per-slot maxima stay tight in every direction
    stats = {}
    for nm in ("s", "d", "c"):
        dsort = np.zeros(nb * P, np.int64)
        dsort[:n_t] = deg[nm][norder]
        per_blk = dsort.reshape(nb, P)
        stats[nm] = (per_blk.max(axis=1), per_blk.sum(axis=1))
    bsort = np.lexsort((stats["c"][1], stats["c"][0],
                        stats["d"][1], stats["d"][0],
                        stats["s"][1], stats["s"][0]))
    q = np.arange(nb)
    core_of = np.empty(nb, np.int64)
    slot_of = np.empty(nb, np.int64)
    core_of[bsort] = q % NCORE
    slot_of[bsort] = q // NCORE
    node_core = core_of[node_gblk]
    node_slot = slot_of[node_gblk]
    node_col = node_core * cfg.shard + node_slot * P + node_loc

    # per-(slot, dir): identity budget (128*maxdeg) vs one-hot budget
    # (ceil128 of max cell count); the cheaper mode wins.
    budgets, modes = {}, {}
    for nm in ("s", "d", "c"):
        md, ct = stats[nm]
        blkmax = md[bsort].reshape(nblk, NCORE).max(axis=1)
        blkcnt = ct[bsort].reshape(nblk, NCORE).max(axis=1)
        B_id = blkmax * P
        B_oh = ((blkcnt + P - 1) // P) * P
        use_id = B_id <= B_oh + P
        budgets[nm] = np.where(use_id, B_id, B_oh)
        modes[nm] = use_id
    Bs, Bd, Bc = budgets["s"], budgets["d"], budgets["c"]
    Ms, Md, Mc = modes["s"], modes["d"], modes["c"]

    off = np.zeros(nblk, np.int64)
    acc = 0
    for blk in range(nblk):
        off[blk] = acc
        acc += Bs[blk] + Bd[blk] + Bc[blk]
    S = int(acc)
    T = S // P
    reg_off = {"s": np.zeros(nblk, np.int64), "d": Bs.copy(),
               "c": (Bs + Bd).copy()}

    dirs = {
        "s": (ett[1], ett[0], xtT),
        "d": (ett[0], ett[1], xtT),
        "c": (ecd, ecs, xcT),
    }

    in_maps = [dict(shared) for _ in range(NCORE)]
    xrawT = np.zeros((P, cfg.nt_pad), BF16)
    xrawT[:, node_col] = xt.T.astype(BF16)
    for k in range(NCORE):
        in_maps[k]["xTm"] = np.ascontiguousarray(
            xrawT[:, k * cfg.shard:(k + 1) * cfg.shard])

    # per-edge placement
    place = {}
    for nm, (key, gnode, srcT) in dirs.items():
        use_id = modes[nm]
        order = np.argsort(key, kind="stable")
        key_s = key[order]
        starts = np.concatenate(
            [[0], np.cumsum(np.bincount(key_s, minlength=n_t))[:-1]])
        rank = np.arange(len(key_s)) - starts[key_s]
        v = (1.0 / np.maximum(deg[nm], 1))[key_s].astype(F32)
        slot_blk = node_slot[key_s]
        loc = node_loc[key_s]
        # identity cells: slot by (rank, dst local)
        slot = rank * P + loc
        # one-hot cells: slot by arrival order within the (core, slot) cell
        ohsel = ~use_id[slot_blk]
        if ohsel.any():
            cell = (node_core[key_s] * nblk + slot_blk)
            cord = np.argsort(cell[ohsel], kind="stable")
            cell_o = cell[ohsel][cord]
            cstarts = np.concatenate(
                [[0], np.cumsum(np.bincount(cell_o, minlength=NCORE * nblk))[:-1]])
            cpos = np.arange(len(cell_o)) - cstarts[cell_o]
            tmp = np.empty(ohsel.sum(), np.int64)
            tmp[cord] = cpos
            slot[ohsel] = tmp
        slot = off[slot_blk] + reg_off[nm][slot_blk] + slot
        place[nm] = (order, key_s, slot, v, gnode, loc)

    for k in range(NCORE):
        stream = np.zeros((P, S), F32)
        dlf = np.full(S, -1.0, F32)
        for nm, (order, key_s, slot, v, gnode, loc) in place.items():
            sel = node_core[key_s] == k
            o = order[sel]
            stream[:, slot[sel]] = dirs[nm][2][:, gnode[o]] * v[sel][None, :]
            dlf[slot[sel]] = loc[sel]
        in_maps[k]["stream"] = stream.astype(BF16)
        in_maps[k]["dl"] = np.ascontiguousarray(
            dlf.reshape(T, P).T.astype(BF16))

    bkey = (tuple(Bs.tolist()), tuple(Bd.tolist()), tuple(Bc.tolist()),
            tuple(bool(x) for x in Ms), tuple(bool(x) for x in Md),
            tuple(bool(x) for x in Mc))
    return in_maps, bkey, node_col


def run(inputs, cfg: Cfg, trace=False, tmpdir=None, trace_cores=None):
    in_maps, bkey, node_col = preprocess(inputs, cfg)
    if bkey not in _prog_cache:
        _prog_cache[bkey] = build_program(cfg, bkey)
    nc = _prog_cache[bkey]
    res = bass_utils.run_bass_kernel_spmd(nc, in_maps, core_ids=list(range(NCORE)),
                                          trace=trace, tmpdir=tmpdir,
                                          trace_cores=trace_cores)
    outT = np.concatenate([res.results[k]["outT"] for k in range(NCORE)], axis=1)
    out = outT[:, node_col].T.astype(F32)
    return out, res


def kernel(**inputs) -> np.ndarray:
    out, _ = run(inputs, FULL, trace=False)
    return out


# revision 56
# speedup vs baseline: 1.0390x; 1.0258x over previous
"""Trainium2 Bass kernel for nn_HeteroForecastSageConv.

Strategy (8 NeuronCores, SPMD, degree-grouped edge streams):
 - Target (destination) nodes are relabeled by lexsorting on their
   (s2d, d2s, ct) in-degrees and chunking 128 consecutive nodes per block;
   blocks are dealt round-robin over the 8 cores.  Within most blocks the
   per-direction degree is constant, so that (block, direction) cell lays
   its edges out as full 128-wide tiles where slot j of tile r holds the
   r-th edge of destination j: the routing matrix is the identity and the
   segment sum is a plain PSUM accumulation of feature-major tiles
   (pretransform with stationary Wp at N<=512).  Cells with mixed degrees
   (degree-class boundaries) fall back to one-hot routing: node-major
   pretransform (stream tile as the stationary operand) plus a DVE-built
   one-hot matmul.  The cheaper budget wins per cell.
 - For each core the host materializes the source feature stream (raw
   input features, feature-major bf16, pre-scaled by 1/deg(dst) and
   shifted by bp @ Wp^-1 so the pretransform bias survives), read strictly
   sequentially by large DMAs.  No gathers, no dynamic descriptors.
 - Epilogue per 4 blocks (feature-major, alpha/hetero weights folded):
       mid = relu(w1^T x_t + ws^T aggS + wd^T aggD + wc^T aggC + bmid)
       out = wo^T mid + bout
Math (alpha = 0.5, folded on host):
  w1 = 0.5 W_self + 0.5 W_ct_r + I,  ws = 0.25 W_s2d, wd = 0.25 W_d2s,
  wc = 0.5 W_ct_l, bmid = 0.5 b_self + 0.25 b_s2d + 0.25 b_d2s + 0.5 b_ct_l
"""
import sys
import dataclasses

sys.path.insert(0, "/opt/trn_rl_repo")

import numpy as np
import ml_dtypes

import concourse.bass as bass
import concourse.bacc as bacc
import concourse.mybir as mybir
import concourse.tile as tile
from concourse import bass_utils

BF16 = ml_dtypes.bfloat16
F32 = np.float32
NCORE = 8
P = 128


@dataclasses.dataclass(frozen=True)
class Cfg:
    n_t: int = 100000
    n_c: int = 20000
    shard: int = 12800       # target nodes per core (multiple of 128)
    chunk_t: int = 128       # stream tiles per DMA chunk (128 tiles = 4 MB)
    run: int = 4             # max tiles per pretransform batch (1 PSUM bank)
    ogrp: int = 8            # output blocks per DMA
    egrp: int = 4            # epilogue batch (blocks)

    @property
    def nt_pad(self):
        return self.shard * NCORE

    @property
    def nblk(self):
        return self.shard // P


FULL = Cfg()

_prog_cache = {}


def _tiles_of(budgets, modes):
    """Flatten per-(block, dir) budgets into the static tile schedule."""
    Bs, Bd, Bc = budgets
    Ms, Md, Mc = modes
    tiles = []  # (blk, reg, is_ct, is_id, reg_first, reg_last, blk_last)
    for blk in range(len(Bs)):
        ccs = [Bs[blk] // P, Bd[blk] // P, Bc[blk] // P]
        ids = [Ms[blk], Md[blk], Mc[blk]]
        tot = sum(ccs)
        seen = 0
        for reg, cc in enumerate(ccs):
            for j in range(cc):
                seen += 1
                tiles.append((blk, reg, reg == 2, bool(ids[reg]), j == 0,
                              j == cc - 1, seen == tot))
    return tiles


def _chunks_of(T, cfg):
    """DMA chunk schedule: a few small leading chunks so compute starts
    early, then full-size chunks."""
    sched = []
    t = 0
    for n in (16, 16, 32, 64):
        if t >= T:
            break
        n = min(n, T - t)
        sched.append((t, n))
        t += n
    while t < T:
        n = min(cfg.chunk_t, T - t)
        sched.append((t, n))
        t += n
    return sched


_V7_COMPAT = True  # plain 128-tile chunks, no pre-issue (measured fastest)


def _chunks_plain(T, cfg):
    sched = []
    t = 0
    while t < T:
        n = min(cfg.chunk_t, T - t)
        sched.append((t, n))
        t += n
    return sched


def _groups_of(tiles, cfg, chunk_starts):
    """Batches of consecutive tiles: same mode, same weight (for identity
    runs), <= cfg.run tiles, never crossing a DMA chunk boundary."""
    groups = []  # (t0, n)
    t = 0
    while t < len(tiles):
        is_ct, is_id = tiles[t][2], tiles[t][3]
        n = 1
        while (n < cfg.run and t + n < len(tiles)
               and tiles[t + n][3] == is_id
               and (not is_id or tiles[t + n][2] == is_ct)
               and (t + n) not in chunk_starts):
            n += 1
        groups.append((t, n))
        t += n
    return groups


def build_program(cfg: Cfg, key):
    budgets, modes = key[:3], key[3:]
    Bs, Bd, Bc = budgets
    nblk = cfg.nblk
    tiles = _tiles_of(budgets, modes)
    chunks = (_chunks_plain if _V7_COMPAT else _chunks_of)(len(tiles), cfg)
    chunk_starts = {t0: (t0, n) for (t0, n) in chunks}
    groups = _groups_of(tiles, cfg, chunk_starts)
    T = len(tiles)
    S = T * P
    dt = mybir.dt
    AF = mybir.ActivationFunctionType
    OP = mybir.AluOpType

    nc = bacc.Bacc("TRN2", target_bir_lowering=False, debug=False)

    def din(name, shape, d):
        return nc.dram_tensor(name, shape, d, kind="ExternalInput")

    t_xTm = din("xTm", [P, cfg.shard], dt.bfloat16)
    t_stream = din("stream", [P, S], dt.bfloat16)
    t_dl = din("dl", [P, max(T, 1)], dt.bfloat16)
    t_wpt = din("wpt", [P, P], dt.bfloat16)
    t_wpc = din("wpc", [P, P], dt.bfloat16)
    t_bpt = din("bpt", [P, 1], dt.float32)
    t_w1 = din("w1", [P, P], dt.bfloat16)
    t_ws = din("ws", [P, P], dt.bfloat16)
    t_wd = din("wd", [P, P], dt.bfloat16)
    t_wc = din("wc", [P, P], dt.bfloat16)
    t_wo = din("wo", [P, P], dt.bfloat16)
    t_bmid = din("bmid", [P, 1], dt.float32)
    t_bout = din("bout", [P, 1], dt.float32)
    t_ident = din("ident", [P, P], dt.bfloat16)
    t_iota = din("iota", [P, P], dt.bfloat16)
    t_out = nc.dram_tensor("outT", [P, cfg.shard], dt.bfloat16, kind="ExternalOutput")

    with tile.TileContext(nc) as tc:
        with tc.tile_pool(name="persist", bufs=1) as pp, \
             tc.tile_pool(name="ch", bufs=2) as chp, \
             tc.tile_pool(name="rr", bufs=3) as rrp, \
             tc.tile_pool(name="ohp", bufs=3) as ohp, \
             tc.tile_pool(name="agp", bufs=2) as agp, \
             tc.tile_pool(name="mip", bufs=2) as mip, \
             tc.tile_pool(name="ogp", bufs=2) as ogp, \
             tc.tile_pool(name="psP", bufs=3, space="PSUM") as psP, \
             tc.tile_pool(name="psA", bufs=2, space="PSUM") as psA, \
             tc.tile_pool(name="psM", bufs=1, space="PSUM") as psM:
            def load(t, shape, d):
                s = pp.tile(shape, d, name=f"sb_{t.name}")
                nc.sync.dma_start(s[:], t.ap())
                return s

            sb_wpt = load(t_wpt, [P, P], dt.bfloat16)
            sb_wpc = load(t_wpc, [P, P], dt.bfloat16)
            sb_bpt = load(t_bpt, [P, 1], dt.float32)
            sb_w1 = load(t_w1, [P, P], dt.bfloat16)
            sb_ws = load(t_ws, [P, P], dt.bfloat16)
            sb_wd = load(t_wd, [P, P], dt.bfloat16)
            sb_wc = load(t_wc, [P, P], dt.bfloat16)
            sb_wo = load(t_wo, [P, P], dt.bfloat16)
            sb_bmid = load(t_bmid, [P, 1], dt.float32)
            sb_bout = load(t_bout, [P, 1], dt.float32)
            sb_ident = load(t_ident, [P, P], dt.bfloat16)
            sb_iota = load(t_iota, [P, P], dt.bfloat16)
            sb_dl = load(t_dl, [P, max(T, 1)], dt.bfloat16)

            # pre-issue the leading stream chunks so compute starts early
            chunk_tiles = {}
            if not _V7_COMPAT:
                for ci, (ct0, cn) in enumerate(chunks[:4]):
                    cs = chp.tile([P, cn * P], dt.bfloat16,
                                  name="chunk", tag=f"chunkp{ci}")
                    nc.sync.dma_start(cs[:],
                                      t_stream.ap()[:, ct0 * P:(ct0 + cn) * P])
                    chunk_tiles[ct0] = cs

            sb_xTm = load(t_xTm, [P, cfg.shard], dt.bfloat16)
            xt_sb = pp.tile([P, cfg.shard], dt.bfloat16)

            # ---- own-shard pretransform (feature-major, stationary Wp_t) ----
            for st in range(0, cfg.shard, 512):
                ps = psP.tile([P, cfg.run * P], dt.float32, name="pre", tag="pre")
                nc.tensor.matmul(ps[:], lhsT=sb_wpt[:],
                                 rhs=sb_xTm[:, st:st + 512],
                                 start=True, stop=True)
                nc.scalar.activation(xt_sb[:, st:st + 512], ps[:],
                                     AF.Relu, bias=sb_bpt[:, 0:1])

            # ---- main stream loop ----
            if True:

                chunk_sb = None
                agg_ps = [None]
                og = [None]
                eng_i = [0]
                next_epi = [0]
                epi_q = []
                EG = cfg.egrp

                def flush_epi():
                    if not epi_q:
                        return
                    q = epi_q[:]
                    epi_q.clear()
                    g = len(q)
                    b0 = q[0][0]
                    ps_mo = psM.tile([P, 2 * EG * P], dt.float32,
                                     name="mo", tag="mo")
                    ps_mid = ps_mo[:, 0:EG * P]
                    ps_out = ps_mo[:, EG * P:2 * EG * P]
                    nc.tensor.matmul(ps_mid[:, 0:g * P], lhsT=sb_w1[:],
                                     rhs=xt_sb[:, b0 * P:(b0 + g) * P],
                                     start=True, stop=False)
                    for i, (blk, sb_agg) in enumerate(q):
                        terms = []
                        if Bs[blk]:
                            terms.append((sb_ws, sb_agg[:, 0:P]))
                        if Bd[blk]:
                            terms.append((sb_wd, sb_agg[:, P:2 * P]))
                        if Bc[blk]:
                            terms.append((sb_wc, sb_agg[:, 2 * P:3 * P]))
                        for k, (wsb, rhs) in enumerate(terms):
                            nc.tensor.matmul(ps_mid[:, i * P:(i + 1) * P],
                                             lhsT=wsb[:], rhs=rhs, start=False,
                                             stop=(k == len(terms) - 1),
                                             skip_group_check=True)
                    sb_mid = mip.tile([P, EG * P], dt.bfloat16,
                                      name="smid", tag="smid")
                    nc.scalar.activation(sb_mid[:, 0:g * P], ps_mid[:, 0:g * P],
                                         AF.Relu, bias=sb_bmid[:, 0:1])
                    nc.tensor.matmul(ps_out[:, 0:g * P], lhsT=sb_wo[:],
                                     rhs=sb_mid[:, 0:g * P], start=True, stop=True)
                    if b0 % cfg.ogrp == 0:
                        og[0] = ogp.tile([P, cfg.ogrp * P], dt.bfloat16,
                                         name="og", tag="og")
                    oo = (b0 % cfg.ogrp) * P
                    nc.scalar.activation(og[0][:, oo:oo + g * P],
                                         ps_out[:, 0:g * P],
                                         AF.Identity, bias=sb_bout[:, 0:1])
                    bl = b0 + g - 1
                    if bl % cfg.ogrp == cfg.ogrp - 1 or bl == nblk - 1:
                        g0 = (bl // cfg.ogrp) * cfg.ogrp
                        gn = bl - g0 + 1
                        nc.sync.dma_start(t_out.ap()[:, g0 * P:(g0 + gn) * P],
                                          og[0][:, :gn * P])

                def enqueue_epi(blk, sb_agg):
                    epi_q.append((blk, sb_agg))
                    if len(epi_q) == EG:
                        flush_epi()

                def finish_block(blk):
                    used = [(0, Bs[blk]), (1, Bd[blk]), (2, Bc[blk])]
                    sb_agg = agp.tile([P, 3 * P], dt.bfloat16,
                                      name="sagg", tag="sagg")
                    runs_ = []
                    for reg, B in used:
                        if not B:
                            continue
                        if runs_ and runs_[-1][1] == reg:
                            runs_[-1] = (runs_[-1][0], reg + 1)
                        else:
                            runs_.append((reg, reg + 1))
                    for a, b in runs_:
                        if blk % 2 == 0:
                            nc.vector.tensor_copy(sb_agg[:, a * P:b * P],
                                                  agg_ps[0][:, a * P:b * P])
                        else:
                            nc.scalar.copy(sb_agg[:, a * P:b * P],
                                           agg_ps[0][:, a * P:b * P])
                    while next_epi[0] < blk:
                        enqueue_epi(next_epi[0], sb_agg)
                        next_epi[0] += 1
                    enqueue_epi(blk, sb_agg)
                    next_epi[0] = blk + 1

                def seg_mm(i, r_ap, oh):
                    blk, reg, _, is_id, first, last, blk_last = tiles[i]
                    if agg_ps[0] is None:
                        agg_ps[0] = psA.tile([P, 3 * P], dt.float32,
                                             name="agg", tag="agg")
                    if is_id:
                        nc.tensor.matmul(agg_ps[0][:, reg * P:(reg + 1) * P],
                                         lhsT=sb_ident[:], rhs=r_ap,
                                         start=first, stop=last)
                    else:
                        nc.tensor.matmul(agg_ps[0][:, reg * P:(reg + 1) * P],
                                         lhsT=r_ap, rhs=oh,
                                         start=first, stop=last)
                    if blk_last:
                        finish_block(blk)
                        agg_ps[0] = None

                cur_c0 = -1
                for (t0, n) in groups:
                    if t0 in chunk_starts:
                        ct0, cn = chunk_starts[t0]
                        if ct0 in chunk_tiles:
                            chunk_sb = chunk_tiles[ct0]
                        else:
                            chunk_sb = chp.tile([P, cfg.chunk_t * P],
                                                dt.bfloat16,
                                                name="chunk", tag="chunk")
                            nc.sync.dma_start(
                                chunk_sb[:, :cn * P],
                                t_stream.ap()[:, ct0 * P:(ct0 + cn) * P])
                        cur_c0 = ct0

                    is_id = tiles[t0][3]
                    off = (t0 - cur_c0) * P
                    ps_run = psP.tile([P, cfg.run * P], dt.float32,
                                      name="pre", tag="pre")
                    if is_id:
                        # feature-major pretransform, stationary Wp
                        is_ct = tiles[t0][2]
                        nc.tensor.matmul(ps_run[:, 0:n * P],
                                         lhsT=(sb_wpc if is_ct else sb_wpt)[:],
                                         rhs=chunk_sb[:, off:off + n * P],
                                         start=True, stop=True)
                    else:
                        # node-major pretransform, stream tiles stationary
                        for i in range(n):
                            nc.tensor.matmul(
                                ps_run[:, i * P:(i + 1) * P],
                                lhsT=chunk_sb[:, off + i * P:off + (i + 1) * P],
                                rhs=(sb_wpc if tiles[t0 + i][2] else sb_wpt)[:],
                                start=True, stop=True)
                    r_sb = rrp.tile([P, cfg.run * P], dt.bfloat16,
                                    name="r", tag="r")
                    if eng_i[0] % 2 == 0:
                        nc.vector.tensor_scalar_max(r_sb[:, 0:n * P],
                                                    ps_run[:, 0:n * P], 0.0)
                    else:
                        nc.scalar.activation(r_sb[:, 0:n * P],
                                             ps_run[:, 0:n * P], AF.Relu)
                    eng_i[0] += 1

                    oh = None
                    if not is_id:
                        oh = ohp.tile([P, cfg.run, P], dt.bfloat16,
                                      name="oh", tag="oh")
                        nc.vector.tensor_tensor(
                            out=oh[:, :n, :],
                            in0=sb_iota[:].unsqueeze(1).to_broadcast([P, n, P]),
                            in1=sb_dl[:, t0:t0 + n].unsqueeze(2)
                                .to_broadcast([P, n, P]),
                            op=OP.is_equal)

                    for i in range(n):
                        seg_mm(t0 + i, r_sb[:, i * P:(i + 1) * P],
                               oh[:, i, :] if oh is not None else None)

                while next_epi[0] < nblk:
                    enqueue_epi(next_epi[0], None)
                    next_epi[0] += 1
                flush_epi()

    nc.compile()
    return nc


def _solve_shift(W, b):
    """delta s.t. delta @ W == b (for folding the pretransform bias into x)."""
    if not np.any(b):
        return np.zeros_like(b)
    try:
        d = np.linalg.solve(W.T.astype(np.float64), b.astype(np.float64))
    except np.linalg.LinAlgError:
        d = np.linalg.lstsq(W.T.astype(np.float64), b.astype(np.float64),
                            rcond=None)[0]
    assert np.allclose(d @ W.astype(np.float64), b, atol=1e-4), \
        "pretransform weight not invertible; bias fold failed"
    return d.astype(F32)


def preprocess(inputs, cfg: Cfg):
    xt = np.asarray(inputs["x_target"], F32)
    xc = np.asarray(inputs["x_context"], F32)
    ett = np.asarray(inputs["edge_tt"]).astype(np.int64)
    ecs = np.asarray(inputs["edge_ct_src"]).astype(np.int64)
    ecd = np.asarray(inputs["edge_ct_dst"]).astype(np.int64)
    n_t = xt.shape[0]
    nblk = cfg.nblk
    nb = NCORE * nblk

    Wp_t = np.asarray(inputs["Wp_t"], F32)
    Wp_c = np.asarray(inputs["Wp_c"], F32)
    bp_t = np.asarray(inputs["bp_t"], F32)
    bp_c = np.asarray(inputs["bp_c"], F32)

    xtT = (xt + _solve_shift(Wp_t, bp_t)).T.copy()
    xcT = (xc + _solve_shift(Wp_c, bp_c)).T.copy()

    W_self = np.asarray(inputs["W_self"], F32)
    W_ct_r = np.asarray(inputs["W_ct_r"], F32)
    w1 = 0.5 * W_self + 0.5 * W_ct_r + np.eye(P, dtype=F32)
    ws = 0.25 * np.asarray(inputs["W_s2d"], F32)
    wd = 0.25 * np.asarray(inputs["W_d2s"], F32)
    wc = 0.5 * np.asarray(inputs["W_ct_l"], F32)
    wo = np.asarray(inputs["W_out"], F32)
    bmid = (0.5 * np.asarray(inputs["b_self"], F32)
            + 0.25 * np.asarray(inputs["b_s2d"], F32)
            + 0.25 * np.asarray(inputs["b_d2s"], F32)
            + 0.5 * np.asarray(inputs["b_ct_l"], F32))
    bout = np.asarray(inputs["b_out"], F32)

    shared = {
        "wpt": np.ascontiguousarray(Wp_t.astype(BF16)),
        "wpc": np.ascontiguousarray(Wp_c.astype(BF16)),
        "bpt": bp_t.reshape(P, 1),
        "w1": w1.astype(BF16), "ws": ws.astype(BF16), "wd": wd.astype(BF16),
        "wc": wc.astype(BF16), "wo": wo.astype(BF16),
        "bmid": bmid.reshape(P, 1), "bout": bout.reshape(P, 1),
        "ident": np.eye(P, dtype=F32).astype(BF16),
        "iota": np.ascontiguousarray(
            np.broadcast_to(np.arange(P, dtype=F32), (P, P)).astype(BF16)),
    }

    # degree-grouped relabeling (see module docstring)
    deg = {
        "s": np.bincount(ett[1], minlength=n_t),
        "d": np.bincount(ett[0], minlength=n_t),
        "c": np.bincount(ecd, minlength=n_t),
    }
    # within (s,d)-degree classes use a deterministic random order so each
    # block samples the ct-degree distribution evenly (ct always one-hot)
    rnd = np.random.RandomState(0).permutation(n_t)
    norder = np.lexsort((rnd, deg["d"], deg["s"]))
    pos = np.arange(n_t)
    node_gblk = np.empty(n_t, np.int64)
    node_loc = np.empty(n_t, np.int64)
    node_gblk[norder] = pos // P
    node_loc[norder] = pos % P

    # per-global-block stats, then group 8 blocks with similar stats into
    # each (slot x 8 cores) so per-slot maxima stay tight in every direction
    stats = {}
    for nm in ("s", "d", "c"):
        dsort = np.zeros(nb * P, np.int64)
        dsort[:n_t] = deg[nm][norder]
        per_blk = dsort.reshape(nb, P)
        stats[nm] = (per_blk.max(axis=1), per_blk.sum(axis=1))
    # quantize count keys to the 128-col budget quantum so the low-priority
    # ct-count key still gets to group similar blocks together
    q128 = lambda x: (x + P - 1) // P
    bsort = np.lexsort((stats["c"][1],
                        q128(stats["d"][1]), stats["d"][0],
                        q128(stats["s"][1]), stats["s"][0]))
    q = np.arange(nb)
    core_of = np.empty(nb, np.int64)
    slot_of = np.empty(nb, np.int64)
    core_of[bsort] = q % NCORE
    slot_of[bsort] = q // NCORE
    node_core = core_of[node_gblk]
    node_slot = slot_of[node_gblk]
    node_col = node_core * cfg.shard + node_slot * P + node_loc

    # per-(slot, dir): identity budget (128*maxdeg) vs one-hot budget
    # (ceil128 of max cell count); the cheaper mode wins.
    budgets, modes = {}, {}
    for nm in ("s", "d", "c"):
        md, ct = stats[nm]
        blkmax = md[bsort].reshape(nblk, NCORE).max(axis=1)
        blkcnt = ct[bsort].reshape(nblk, NCORE).max(axis=1)
        B_id = blkmax * P
        B_oh = ((blkcnt + P - 1) // P) * P
        use_id = B_id <= B_oh + P
        budgets[nm] = np.where(use_id, B_id, B_oh)
        modes[nm] = use_id
    Bs, Bd, Bc = budgets["s"], budgets["d"], budgets["c"]
    Ms, Md, Mc = modes["s"], modes["d"], modes["c"]

    off = np.zeros(nblk, np.int64)
    acc = 0
    for blk in range(nblk):
        off[blk] = acc
        acc += Bs[blk] + Bd[blk] + Bc[blk]
    S = int(acc)
    T = S // P
    reg_off = {"s": np.zeros(nblk, np.int64), "d": Bs.copy(),
               "c": (Bs + Bd).copy()}

    dirs = {
        "s": (ett[1], ett[0], xtT),
        "d": (ett[0], ett[1], xtT),
        "c": (ecd, ecs, xcT),
    }

    in_maps = [dict(shared) for _ in range(NCORE)]
    xrawT = np.zeros((P, cfg.nt_pad), BF16)
    xrawT[:, node_col] = xt.T.astype(BF16)
    for k in range(NCORE):
        in_maps[k]["xTm"] = np.ascontiguousarray(
            xrawT[:, k * cfg.shard:(k + 1) * cfg.shard])

    # per-edge placement
    place = {}
    for nm, (key, gnode, srcT) in dirs.items():
        use_id = modes[nm]
        order = np.argsort(key, kind="stable")
        key_s = key[order]
        starts = np.concatenate(
            [[0], np.cumsum(np.bincount(key_s, minlength=n_t))[:-1]])
        rank = np.arange(len(key_s)) - starts[key_s]
        v = (1.0 / np.maximum(deg[nm], 1))[key_s].astype(F32)
        slot_blk = node_slot[key_s]
        loc = node_loc[key_s]
        # identity cells: slot by (rank, dst local)
        slot = rank * P + loc
        # one-hot cells: slot by arrival order within the (core, slot) cell
        ohsel = ~use_id[slot_blk]
        if ohsel.any():
            cell = (node_core[key_s] * nblk + slot_blk)
            cord = np.argsort(cell[ohsel], kind="stable")
            cell_o = cell[ohsel][cord]
            cstarts = np.concatenate(
                [[0], np.cumsum(np.bincount(cell_o, minlength=NCORE * nblk))[:-1]])
            cpos = np.arange(len(cell_o)) - cstarts[cell_o]
            tmp = np.empty(ohsel.sum(), np.int64)
            tmp[cord] = cpos
            slot[ohsel] = tmp
        slot = off[slot_blk] + reg_off[nm][slot_blk] + slot
        place[nm] = (order, key_s, slot, v, gnode, loc)

    for k in range(NCORE):
        stream = np.zeros((P, S), F32)
        dlf = np.full(S, -1.0, F32)
        for nm, (order, key_s, slot, v, gnode, loc) in place.items():
            sel = node_core[key_s] == k
            o = order[sel]
            stream[:, slot[sel]] = dirs[nm][2][:, gnode[o]] * v[sel][None, :]
            dlf[slot[sel]] = loc[sel]
        in_maps[k]["stream"] = stream.astype(BF16)
        in_maps[k]["dl"] = np.ascontiguousarray(
            dlf.reshape(T, P).T.astype(BF16))

    bkey = (tuple(Bs.tolist()), tuple(Bd.tolist()), tuple(Bc.tolist()),
            tuple(bool(x) for x in Ms), tuple(bool(x) for x in Md),
            tuple(bool(x) for x in Mc))
    return in_maps, bkey, node_col


def run(inputs, cfg: Cfg, trace=False, tmpdir=None, trace_cores=None):
    in_maps, bkey, node_col = preprocess(inputs, cfg)
    if bkey not in _prog_cache:
        _prog_cache[bkey] = build_program(cfg, bkey)
    nc = _prog_cache[bkey]
    res = bass_utils.run_bass_kernel_spmd(nc, in_maps, core_ids=list(range(NCORE)),
                                          trace=trace, tmpdir=tmpdir,
                                          trace_cores=trace_cores)
    outT = np.concatenate([res.results[k]["outT"] for k in range(NCORE)], axis=1)
    out = outT[:, node_col].T.astype(F32)
    return out, res


def kernel(**inputs) -> np.ndarray:
    out, _ = run(inputs, FULL, trace=False)
    return out
